# revision 1
# baseline (speedup 1.0000x reference)
"""RGCN (mean-aggr) Trainium2 kernel, 8-core SPMD, dst-sharded. v3.

Baseline two-phase gather structure (HW-proven primitives only), plus:
  - bf16 datapath (x, staging, weights, means, output): half DMA, 4x matmul.
  - Two-class LPT dst->tile bins: per 28-tile sweep, 16 heavy tiles (cap 128,
    own gather column) and 12 light tiles (cap 64); light tile pairs share one
    gather column and one 256-wide one-hot segment matmul (full 128-token
    contraction). Phase-A buckets dedupe repeated src rows.
  - Phase A (per sub x src-window) dma_gather writes B_d contiguously
    (p-major), phase B per-sweep dma_gather re-reads in tile-major order.
  - agg PSUM split A/B (3+4 banks) drained in parallel on DVE/Act; transform
    split accordingly; bias drain alternates engines; x^T preloaded to SBUF.
Output is out^T (bf16) per core in permuted dst order; host inverts.
"""

import heapq

import numpy as np
import ml_dtypes

BF16 = ml_dtypes.bfloat16

P = 128
N_NODES = 100000
N_EDGES = 600000
DIM = 128
NUM_RELS = 8
NCORES = 8

TILE_DST = 16
TILE_SLOTS = TILE_DST * NUM_RELS          # 128
NTILES = 784                              # per core
CW = NTILES * TILE_DST                    # 12544
NBINS = NCORES * NTILES                   # 6272
NSUB = 2                                  # dst subranges per core (phase A)
TPS = NTILES // NSUB                      # 392 tiles per sub
NQ = 4                                    # src windows
QW = 25088                                # src window width
SWEEP_TILES = 28
NSWEEPS = NTILES // SWEEP_TILES           # 28
HEAVY_T = 16                              # heavy slots per sweep (cap 128)
LIGHT_T = SWEEP_TILES - HEAVY_T           # light slots (cap 64, share columns)
NCOLS_SWEEP = HEAVY_T + LIGHT_T // 2      # gather columns per sweep (22)
NHEAVY = NSWEEPS * HEAVY_T                # per core
NLIGHT = NSWEEPS * LIGHT_T
SWEEP_SLOTS = SWEEP_TILES * TILE_SLOTS    # 3584
SWEEP_DST = SWEEP_TILES * TILE_DST        # 448
SPLIT_TILES = 12                          # aggA (3 PSUM banks)
SPLIT_SLOTS = SPLIT_TILES * TILE_SLOTS    # 1536
SPLIT_DST = SPLIT_TILES * TILE_DST        # 192
RESTB_SLOTS = SWEEP_SLOTS - SPLIT_SLOTS   # 2048 (4 banks)
SWEEPS_PER_SUB = TPS // SWEEP_TILES       # 14

_compiled = None


def _wrap16(idx_i16):
    n = len(idx_i16)
    w = idx_i16.reshape(n // 16, 16).T
    return np.ascontiguousarray(np.tile(w, (8, 1)))


def _build_program(CAPA):
    import concourse.bacc as bacc
    import concourse.tile as tile
    from concourse import mybir

    TOTCH = NSWEEPS * NCOLS_SWEEP
    TOTB = TOTCH * P
    AC = CAPA // P                        # chunk columns per (sub,q) bucket
    BROWS = NQ * CAPA + P                 # per-sub B rows (+zero row block)

    nc = bacc.Bacc(None, target_bir_lowering=False, debug=False)
    f32 = mybir.dt.float32
    bf16 = mybir.dt.bfloat16
    i16 = mybir.dt.int16
    i32 = mybir.dt.int32

    xg_d = nc.dram_tensor("xg", [NQ * QW, P], bf16, kind="ExternalInput")
    xT_d = nc.dram_tensor("xT", [P, CW], bf16, kind="ExternalInput")
    wcat_d = nc.dram_tensor("wcat", [P, NUM_RELS * P], bf16, kind="ExternalInput")
    wroot_d = nc.dram_tensor("wroot", [P, P], bf16, kind="ExternalInput")
    bias_d = nc.dram_tensor("bias", [P, 1], f32, kind="ExternalInput")
    gA_d = nc.dram_tensor("gA", [NSUB * NQ, P, CAPA // 16], i16, kind="ExternalInput")
    gB_d = nc.dram_tensor("gB", [P, TOTB // 16], i16, kind="ExternalInput")
    scol_d = nc.dram_tensor("scol", [P, TOTCH], f32, kind="ExternalInput")
    wgt_d = nc.dram_tensor("wgt", [P, TOTCH], f32, kind="ExternalInput")
    outT_d = nc.dram_tensor("outT", [P, CW], bf16, kind="ExternalOutput")

    B_d = [nc.dram_tensor(f"B{s}", [BROWS, P], bf16) for s in range(NSUB)]

    with tile.TileContext(nc) as tc:
        with (
            tc.tile_pool(name="const", bufs=1) as cpool,
            tc.tile_pool(name="stagA", bufs=3) as poolA,
            tc.tile_pool(name="stagB", bufs=4) as poolB,
            tc.tile_pool(name="spool", bufs=16) as spool,
            tc.tile_pool(name="mpool", bufs=4) as mpool,
            tc.tile_pool(name="opool", bufs=4) as opool,
            tc.tile_pool(name="ipool", bufs=4) as ipool,
            tc.tile_pool(name="psA", bufs=1, space="PSUM") as psA,
            tc.tile_pool(name="psO", bufs=1, space="PSUM") as psO,
        ):
            wcat = cpool.tile([P, NUM_RELS * P], bf16)
            wroot = cpool.tile([P, P], bf16)
            biast = cpool.tile([P, 1], f32)
            iota_i = cpool.tile([P, 2 * P], i32)
            iota_f = cpool.tile([P, 2 * P], bf16)
            zrow = cpool.tile([P, P], bf16)
            scolt = cpool.tile([P, TOTCH], f32)
            wgtt = cpool.tile([P, TOTCH], f32)
            xTt = cpool.tile([P, CW], bf16)

            nc.sync.dma_start(out=scolt[:], in_=scol_d[:])
            nc.sync.dma_start(out=wgtt[:], in_=wgt_d[:])
            nc.sync.dma_start(out=wcat[:], in_=wcat_d[:])
            nc.sync.dma_start(out=wroot[:], in_=wroot_d[:])
            nc.sync.dma_start(out=biast[:], in_=bias_d[:])
            nc.sync.dma_start(out=xTt[:], in_=xT_d[:])
            nc.gpsimd.iota(iota_i[:], pattern=[[1, 2 * P]], base=0,
                           channel_multiplier=0)
            nc.vector.tensor_copy(out=iota_f[:], in_=iota_i[:])
            nc.vector.memset(zrow[:], 0.0)

            # ---- Phase A: src-window gathers -> B_s (contiguous p-major) ----
            for s in range(NSUB):
                nc.sync.dma_start(
                    out=B_d[s][NQ * CAPA:NQ * CAPA + P, :], in_=zrow[:])
                for q in range(NQ):
                    gA = ipool.tile([P, CAPA // 16], i16, tag="gA")
                    nc.sync.dma_start(out=gA[:], in_=gA_d[s * NQ + q])
                    stag = poolA.tile([P, AC, P], bf16, tag="stagA")
                    nc.gpsimd.dma_gather(
                        out_ap=stag[:],
                        in_ap=xg_d[QW * q:QW * (q + 1), :],
                        idxs_ap=gA[:],
                        num_idxs=CAPA, num_idxs_reg=CAPA, elem_size=P,
                        single_packet=False)
                    nc.sync.dma_start(
                        out=B_d[s][CAPA * q:CAPA * (q + 1), :].rearrange(
                            "(p a) d -> p a d", p=P),
                        in_=stag[:])

            # ---- Phase B: per-sweep gathers + segment + transform ----
            for s in range(NSWEEPS):
                sub = s // SWEEPS_PER_SUB
                c0 = s * NCOLS_SWEEP
                swtok = NCOLS_SWEEP * P
                gB = ipool.tile([P, swtok // 16], i16, tag="gB")
                nc.sync.dma_start(
                    out=gB[:], in_=gB_d[:, c0 * P // 16:(c0 + NCOLS_SWEEP) * P // 16])
                stag = poolB.tile([P, NCOLS_SWEEP, P], bf16, tag="stagB")
                nc.gpsimd.dma_gather(
                    out_ap=stag[:], in_ap=B_d[sub][:, :], idxs_ap=gB[:],
                    num_idxs=swtok, num_idxs_reg=swtok, elem_size=P,
                    single_packet=False)

                aggA = psA.tile([P, SPLIT_SLOTS], f32, tag="aggA")
                aggB = psA.tile([P, RESTB_SLOTS], f32, tag="aggB")
                for tl in range(SWEEP_TILES):
                    if tl < HEAVY_T:
                        col = c0 + tl
                        if tl < SPLIT_TILES:
                            aggv = aggA[:, tl * TILE_SLOTS:(tl + 1) * TILE_SLOTS]
                        else:
                            tb = tl - SPLIT_TILES
                            aggv = aggB[:, tb * TILE_SLOTS:(tb + 1) * TILE_SLOTS]
                        Sc = spool.tile([P, P], bf16, tag="S")
                        nc.vector.tensor_scalar(
                            out=Sc[:], in0=iota_f[:, 0:P],
                            scalar1=scolt[:, col:col + 1],
                            scalar2=wgtt[:, col:col + 1],
                            op0=mybir.AluOpType.is_equal,
                            op1=mybir.AluOpType.mult)
                        nc.tensor.matmul(
                            out=aggv, lhsT=stag[:, col - c0, :], rhs=Sc[:],
                            start=True, stop=True)
                    elif (tl - HEAVY_T) % 2 == 0:
                        col = c0 + HEAVY_T + (tl - HEAVY_T) // 2
                        tb = tl - SPLIT_TILES
                        aggv = aggB[:, tb * TILE_SLOTS:(tb + 2) * TILE_SLOTS]
                        Sc2 = spool.tile([P, 2 * P], bf16, tag="S2")
                        nc.vector.tensor_scalar(
                            out=Sc2[:], in0=iota_f[:],
                            scalar1=scolt[:, col:col + 1],
                            scalar2=wgtt[:, col:col + 1],
                            op0=mybir.AluOpType.is_equal,
                            op1=mybir.AluOpType.mult)
                        nc.tensor.matmul(
                            out=aggv, lhsT=stag[:, col - c0, :], rhs=Sc2[:],
                            start=True, stop=True)

                meanA = mpool.tile([P, SPLIT_SLOTS], bf16, tag="meanA")
                meanB = mpool.tile([P, RESTB_SLOTS], bf16, tag="meanB")
                if s % 2 == 0:
                    nc.vector.tensor_copy(out=meanA[:], in_=aggA[:])
                else:
                    nc.scalar.activation(
                        out=meanA[:], in_=aggA[:],
                        func=mybir.ActivationFunctionType.Identity)
                nc.scalar.activation(
                    out=meanB[:], in_=aggB[:],
                    func=mybir.ActivationFunctionType.Identity)

                dst0 = s * SWEEP_DST
                outp = psO.tile([P, SWEEP_DST], f32)
                meanA_r = meanA[:].rearrange(
                    "p (dst rel) -> p dst rel", rel=NUM_RELS)
                meanB_r = meanB[:].rearrange(
                    "p (dst rel) -> p dst rel", rel=NUM_RELS)
                for r in range(NUM_RELS):
                    nc.tensor.matmul(
                        out=outp[:, :SPLIT_DST],
                        lhsT=wcat[:, r * P:(r + 1) * P],
                        rhs=meanA_r[:, :, r],
                        start=(r == 0), stop=False)
                nc.tensor.matmul(out=outp[:, :SPLIT_DST], lhsT=wroot[:],
                                 rhs=xTt[:, dst0:dst0 + SPLIT_DST],
                                 start=False, stop=True)
                for r in range(NUM_RELS):
                    nc.tensor.matmul(
                        out=outp[:, SPLIT_DST:],
                        lhsT=wcat[:, r * P:(r + 1) * P],
                        rhs=meanB_r[:, :, r],
                        start=(r == 0), stop=False)
                nc.tensor.matmul(out=outp[:, SPLIT_DST:], lhsT=wroot[:],
                                 rhs=xTt[:, dst0 + SPLIT_DST:dst0 + SWEEP_DST],
                                 start=False, stop=True)
                oT = opool.tile([P, SWEEP_DST], bf16, tag="oT")
                if s % 2 == 0:
                    nc.vector.tensor_scalar_add(
                        out=oT[:], in0=outp[:], scalar1=biast[:, 0:1])
                else:
                    nc.scalar.activation(
                        out=oT[:], in_=outp[:],
                        func=mybir.ActivationFunctionType.Identity,
                        bias=biast[:, 0:1])
                nc.sync.dma_start(out=outT_d[:, dst0:dst0 + SWEEP_DST], in_=oT[:])
    nc.compile()
    return nc


def _balance(cnt_dst):
    """Two-class LPT: top-degree dst into heavy bins (cap 128), rest into
    light bins (cap 64), 16 dst each. Returns bin_of, pos_of, loads; bins
    [0, NHBINS) heavy, rest light."""
    NHBINS = NCORES * NHEAVY
    NLBINS = NCORES * NLIGHT
    order = np.argsort(-cnt_dst, kind="stable")
    bin_of = np.empty(N_NODES, np.int64)
    pos_of = np.empty(N_NODES, np.int64)
    counts = np.zeros(NBINS, np.int32)
    loads = np.zeros(NBINS, np.int64)
    nheavy_dst = NHBINS * TILE_DST
    for part, cap in ((order[:nheavy_dst], P), (order[nheavy_dst:], 64)):
        b0 = 0 if cap == P else NHBINS
        nb = NHBINS if cap == P else NLBINS
        heap = [(0, 0, b0 + b) for b in range(nb)]
        for d in part:
            deg = int(cnt_dst[d])
            load, c, b = heapq.heappop(heap)
            assert loads[b] + deg <= cap, "two-class packing infeasible"
            bin_of[d] = b
            pos_of[d] = counts[b]
            counts[b] += 1
            loads[b] += deg
            if counts[b] < TILE_DST:
                heapq.heappush(heap, (loads[b], counts[b], b))
    return bin_of, pos_of, loads


def _prepare(x, W, W_root, bias, edge_index, edge_type):
    src = np.asarray(edge_index[0], dtype=np.int64)
    dst = np.asarray(edge_index[1], dtype=np.int64)
    rel = np.asarray(edge_type, dtype=np.int64)

    cnt_slot = np.bincount(dst * NUM_RELS + rel, minlength=N_NODES * NUM_RELS)
    w_edge = (1.0 / np.maximum(cnt_slot[dst * NUM_RELS + rel], 1)).astype(np.float32)
    cnt_dst = np.bincount(dst, minlength=N_NODES).astype(np.int64)

    bin_of, pos_of, bin_load = _balance(cnt_dst)
    NHBINS = NCORES * NHEAVY
    # rank heavy and light bins separately; deal to (slot, core)
    tile_of_bin = np.empty(NBINS, np.int64)
    core_of_bin = np.empty(NBINS, np.int64)
    hrank = np.argsort(-bin_load[:NHBINS], kind="stable")
    hslot = np.arange(NHBINS) // NCORES        # 0..NHEAVY-1
    tile_of_bin[hrank] = (hslot // HEAVY_T) * SWEEP_TILES + hslot % HEAVY_T
    core_of_bin[hrank] = np.arange(NHBINS) % NCORES
    lrank = NHBINS + np.argsort(-bin_load[NHBINS:], kind="stable")
    lslot = np.arange(NCORES * NLIGHT) // NCORES
    tile_of_bin[lrank] = ((lslot // LIGHT_T) * SWEEP_TILES + HEAVY_T
                          + lslot % LIGHT_T)
    core_of_bin[lrank] = np.arange(NCORES * NLIGHT) % NCORES

    core_of_dst = core_of_bin[bin_of]
    tile_of_dst = tile_of_bin[bin_of]
    j_of_dst = pos_of

    e_core = core_of_dst[dst]
    e_tile = tile_of_dst[dst]
    e_scol = j_of_dst[dst] * NUM_RELS + rel
    e_sub = e_tile // TPS
    q = src // QW

    # phase A bucket caps (core, sub, q) on UNIQUE src rows, shared across cores
    keyA = (e_core * NSUB + e_sub) * NQ + q
    upairs = np.unique(keyA * (N_NODES + 1) + src)
    bincA = np.bincount(upairs // (N_NODES + 1), minlength=NCORES * NSUB * NQ)
    CAPA = int(-(-bincA.max() // P) * P)
    CAPA = max(CAPA, P)
    AC = CAPA // P

    # two-class column layout (fixed): per tile slot
    tl_all = np.arange(NTILES) % SWEEP_TILES
    sw_all = np.arange(NTILES) // SWEEP_TILES
    is_heavy = tl_all < HEAVY_T
    col_of_tile = np.where(
        is_heavy, sw_all * NCOLS_SWEEP + tl_all,
        sw_all * NCOLS_SWEEP + HEAVY_T + (tl_all - HEAVY_T) // 2)
    pbase_of_tile = np.where(is_heavy, 0, ((tl_all - HEAVY_T) % 2) * 64)
    # light pair member B one-hots target Sc2 columns [128, 256)
    soff_of_tile = np.where(is_heavy, 0, ((tl_all - HEAVY_T) % 2) * P)
    cap_of_tile = np.where(is_heavy, P, 64)
    TOTCH = NSWEEPS * NCOLS_SWEEP
    TOTB = TOTCH * P

    xg = np.zeros((NQ * QW, P), np.float32)
    xg[:N_NODES] = np.asarray(x, np.float32)
    xg = xg.astype(BF16)
    wcat = np.ascontiguousarray(
        np.asarray(W, np.float32).transpose(1, 0, 2).reshape(P, NUM_RELS * P)
    ).astype(BF16)
    wroot = np.ascontiguousarray(np.asarray(W_root, np.float32)).astype(BF16)
    biascol = np.asarray(bias, np.float32).reshape(P, 1)

    order_e = np.lexsort((e_scol, e_tile, e_core))
    in_maps = []
    dst_tables = []
    xnp = np.asarray(x, np.float32)
    for c in range(NCORES):
        sel = order_e[e_core[order_e] == c]
        csrc, cq, csub, cscol, ctile, cw = (
            src[sel], q[sel], e_sub[sel], e_scol[sel], e_tile[sel], w_edge[sel])

        # phase A: bucket by (sub, q); rank within bucket
        keyaq = csub * NQ + cq
        ordA = np.argsort(keyaq, kind="stable")
        gA = np.zeros((NSUB * NQ, P, CAPA // 16), np.int16)
        rankA = np.zeros(len(sel), np.int64)
        for sq in range(NSUB * NQ):
            members = ordA[keyaq[ordA] == sq]
            uniq, inv = np.unique(csrc[members], return_inverse=True)
            n = len(uniq)
            assert n <= CAPA, (n, CAPA)
            rankA[members] = inv
            idx = np.zeros(CAPA, np.int16)
            qbase = QW * (sq % NQ)
            idx[:n] = (uniq - qbase).astype(np.int16)
            gA[sq] = _wrap16(idx)
        # B row for each edge: stag row r=(p,a) written p-major
        brow = CAPA * cq + (rankA % P) * AC + rankA // P

        # phase B tokens: tile-major with per-tile caps
        gB_lin = np.full(TOTB, NQ * CAPA, np.int64)  # default: zero row
        gB_lin += np.arange(TOTB) % P
        scol_lin = np.full(TOTB, -1.0, np.float32)
        wgt_lin = np.zeros(TOTB, np.float32)
        ordT = np.argsort(ctile, kind="stable")
        tcounts = np.bincount(ctile, minlength=NTILES)
        tstart = np.concatenate([[0], np.cumsum(tcounts)])[:-1]
        arangepos = np.empty(len(sel), np.int64)
        arangepos[ordT] = np.arange(len(sel))
        rank_in_tile = arangepos - tstart[ctile]
        assert (rank_in_tile < cap_of_tile[ctile]).all()
        tok = col_of_tile[ctile] * P + pbase_of_tile[ctile] + rank_in_tile
        gB_lin[tok] = brow
        scol_lin[tok] = (cscol + soff_of_tile[ctile]).astype(np.float32)
        wgt_lin[tok] = cw
        assert gB_lin.max() < 32768

        mask = core_of_dst == c
        dst_ids = np.nonzero(mask)[0]
        cols = tile_of_dst[dst_ids] * TILE_DST + j_of_dst[dst_ids]
        dst_table = np.full(CW, -1, np.int64)
        dst_table[cols] = dst_ids
        valid = dst_table >= 0
        xT = np.zeros((P, CW), np.float32)
        xT[:, valid] = xnp[dst_table[valid]].T
        xT = xT.astype(BF16)

        # scol/wgt as [P, TOTCH]: token (p, col) -> p = tok % P, col = tok // P
        scol_arr = np.ascontiguousarray(
            scol_lin.reshape(TOTCH, P).T)
        wgt_arr = np.ascontiguousarray(
            wgt_lin.reshape(TOTCH, P).T)

        in_maps.append({
            "xg": xg,
            "xT": xT,
            "wcat": wcat,
            "wroot": wroot,
            "bias": biascol,
            "gA": gA,
            "gB": _wrap16(gB_lin.astype(np.int16)),
            "scol": scol_arr,
            "wgt": wgt_arr,
        })
        dst_tables.append(dst_table)
    return in_maps, dst_tables, CAPA


LAST_EXEC_NS = None


def kernel(x, W, W_root, bias, edge_index, edge_type):
    global _compiled, LAST_EXEC_NS
    import os
    from concourse.bass_utils import run_bass_kernel_spmd

    in_maps, dst_tables, CAPA = _prepare(
        x, W, W_root, bias, edge_index, edge_type)
    key = CAPA
    if _compiled is None or _compiled[0] != key:
        nc = _build_program(CAPA)
        _compiled = (key, nc)
    nc = _compiled[1]

    trace = bool(int(os.environ.get("BASS_PROFILE", "0")))
    r = run_bass_kernel_spmd(nc, in_maps, list(range(NCORES)), trace=trace)
    if trace and getattr(r, "exec_time_ns", None) is not None:
        LAST_EXEC_NS = r.exec_time_ns
    res = r.results
    out = np.empty((N_NODES, DIM), np.float32)
    for c in range(NCORES):
        outT = np.asarray(res[c]["outT"]).astype(np.float32)
        dt = dst_tables[c]
        valid = dt >= 0
        out[dt[valid]] = outT[:, valid].T
    return out

